# revision 15
# baseline (speedup 1.0000x reference)
"""Multi-head attention (B=2, S=2048, D=1024, H=16 heads, causal) on 8 trn2 cores.

Sharding: heads across cores (2 heads = 128 channels per core).
  - W_q/W_k/W_v column-sharded: each core projects all tokens to its 128 channels.
  - Attention per (batch, head) fully local to a core.
  - W_o row-sharded: each core computes a partial output projection; partials
    are summed on the host (the unshard step), then b_o is added.

Device layout: everything transposed (channels on partitions, tokens on free).
  - Scores computed as S^T blocks [128 k-tok, 512 q-tok] so exp is elementwise
    and the softmax sum comes for free from a ones-column packed next to V^T.
  - V^T is produced by DMA XBAR transpose (no PE/DVE involvement) into a
    per-(k-block, head) 128-column padded slot [V | ones | pad] (XBAR needs
    aligned destinations), so each head's AV lhsT is one contiguous
    65-column slice with the softmax sum landing in PSUM row 64.
  - Causal structure: host inspects the mask and emits only non-empty blocks,
    trimmed to their valid q-window [qlo, qhi); partially-valid q ranges are
    multiplied by a deduplicated 0/1 pattern tile post-exp.
  - Softmax normalization: reciprocal of the sums rows, broadcast across the
    64 partitions of each head via a K=1 outer-product matmul on the PE
    (no DRAM bounce), then one multiply per head straight out of PSUM.
  - Attention columns are emitted one 512-token projection group later than
    their inputs require, so the scheduler always has projection matmuls
    available to hide the exp/normalization latency and keep the PE clock warm.

All matmuls run in bf16 (inputs cast on host) with fp32 PSUM accumulation;
the partial output is returned bf16 and reduced in fp32 on the host.
"""

import sys

import numpy as np

try:
    import concourse.bass as bass  # noqa: F401
except ImportError:  # pragma: no cover
    sys.path.insert(0, "/opt/trn_rl_repo")

import ml_dtypes

import concourse.mybir as mybir
import concourse.tile as tile
from concourse import bacc, bass_utils

P = 128
B, S, D = 2, 2048, 1024
H, DK = 16, 64
N_CORES = 8
HPC = H // N_CORES  # heads per core = 2
CH = HPC * DK  # channels per core = 128
TOK = B * S  # 4096
NKB = S // P  # k-blocks per batch = 16
CW = 512  # q column width
NJ = S // CW  # q columns per batch = 4
NTG = S // CW  # 512-token projection groups per batch = 4
KPG = CW // P  # k-blocks per token group = 4
XC = D // P  # x-dim chunks = 8
MO = D // P  # output-channel chunks = 8
VW = DK + 1  # 65: [V | ones] per head per k-block

BF16 = mybir.dt.bfloat16
F32 = mybir.dt.float32
NPBF16 = ml_dtypes.bfloat16

_BUILD_CACHE = {}


def _analyze_mask(mask):
    """Block plan from the (1,1,S,S) boolean mask (shared across batch/head).

    plan[j] = tuple of (bk, qlo, qhi, mixed) for each k-block with any valid
    entry; [qlo, qhi) is the smallest q-window containing all valid rows,
    mixed = (pat_off, a, w) for a range needing a 0/1 multiply (None if the
    window is fully valid). Patterns are deduplicated and concatenated into
    pats (P, W_total) in [k, q] layout.
    """
    m = np.asarray(mask).reshape(S, S).astype(bool)  # m[q, k]
    pat_index = {}
    pat_list = []
    plan = []
    for j in range(NJ):
        q0 = j * CW
        blocks = []
        for bk in range(NKB):
            sub = m[q0 : q0 + CW, bk * P : (bk + 1) * P]  # (CW q, P k)
            any_row = sub.any(axis=1)
            if not any_row.any():
                continue
            qlo = int(np.argmax(any_row))
            qhi = int(CW - np.argmax(any_row[::-1]))
            valid_all = sub.all(axis=1)
            notfull = ~valid_all
            notfull[:qlo] = False
            notfull[qhi:] = False
            mixed = None
            if notfull.any():
                idx = np.where(notfull)[0]
                a, b_ = int(idx[0]), int(idx[-1]) + 1
                patt = np.ascontiguousarray(sub[a:b_, :].T).astype(np.float32)
                key = (patt.shape[1], patt.tobytes())
                if key not in pat_index:
                    pat_index[key] = len(pat_list)
                    pat_list.append(patt)
                mixed = (pat_index[key], a, b_ - a)
            blocks.append((bk, qlo, qhi, mixed))
        plan.append(tuple(blocks))
    offs = [0]
    for p_ in pat_list:
        offs.append(offs[-1] + p_.shape[1])
    plan2 = []
    for col in plan:
        col2 = []
        for bk, qlo, qhi, mixed in col:
            if mixed is not None:
                pid, a, w = mixed
                mixed = (offs[pid], a, w)
            col2.append((bk, qlo, qhi, mixed))
        plan2.append(tuple(col2))
    if pat_list:
        pat_arr = np.concatenate(pat_list, axis=1)  # (P, W_total)
    else:
        pat_arr = np.ones((P, 1), np.float32)
    return tuple(plan2), pat_arr


def _build(plan, pat_w):
    nc = bacc.Bacc(
        "TRN2",
        target_bir_lowering=False,
        debug=False,
        enable_asserts=True,
        num_devices=N_CORES,
    )
    NTT = B * NTG
    xq = nc.dram_tensor("xq", [NTT, P, XC, CW], BF16, kind="ExternalInput").ap()
    xk = nc.dram_tensor("xk", [NTT, P, XC, CW], BF16, kind="ExternalInput").ap()
    xv = nc.dram_tensor("xv", [NTT, P, XC, CW], BF16, kind="ExternalInput").ap()
    wq = nc.dram_tensor("wq", [D, CH], BF16, kind="ExternalInput").ap()
    wk = nc.dram_tensor("wk", [D, CH], BF16, kind="ExternalInput").ap()
    wv = nc.dram_tensor("wv", [D, CH], BF16, kind="ExternalInput").ap()
    wo = nc.dram_tensor("wo", [CH, D], BF16, kind="ExternalInput").ap()
    bq = nc.dram_tensor("bq", [CH, 1], F32, kind="ExternalInput").ap()
    bk_ = nc.dram_tensor("bk", [CH, 1], F32, kind="ExternalInput").ap()
    bv = nc.dram_tensor("bv", [CH, 1], F32, kind="ExternalInput").ap()
    mpat = nc.dram_tensor("mpat", [P, pat_w], BF16, kind="ExternalInput").ap()
    out = nc.dram_tensor(
        "out", [B * NJ, P, MO, CW], BF16, kind="ExternalOutput"
    ).ap()

    # slot (one per 512-token projection group) at which each attention
    # column runs: one group AFTER its last input group, so projection
    # matmuls are always available to hide softmax latency.
    attn_after = [max((bk for bk, _, _, _ in col), default=0) // KPG for col in plan]
    n_slots = B * NTG
    attn_sched = [[] for _ in range(n_slots + 1)]
    for b in range(B):
        for j in range(NJ):
            s = min(b * NTG + attn_after[j] + 1, n_slots)
            attn_sched[s].append((b, j))

    with tile.TileContext(nc) as tc:
        with (
            tc.tile_pool(name="const", bufs=1) as const,
            tc.tile_pool(name="persist", bufs=1) as persist,
            tc.tile_pool(name="xt", bufs=6) as xtp,
            tc.tile_pool(name="a2", bufs=4) as a2p,
            tc.tile_pool(name="yt", bufs=4) as ytp,
            tc.tile_pool(name="ob", bufs=3) as obp,
            tc.tile_pool(name="small", bufs=3) as small,
            tc.tile_pool(name="pp", bufs=2, space="PSUM") as pp,
            tc.tile_pool(name="s2", bufs=3, space="PSUM") as s2p,
            tc.tile_pool(name="op", bufs=2, space="PSUM") as opp,
            tc.tile_pool(name="sc", bufs=1, space="PSUM") as scp,
        ):
            # ones row for the softmax-scale broadcast outer product:
            # scale[d, q] = ones64[0, d] * rec_h[0, q]
            ones64 = const.tile([1, DK], BF16, tag="ones64")
            nc.gpsimd.memset(ones64[:], 1.0)

            w_sb = {}
            b_sb = {}
            for name, wdram, bdram in (
                ("q", wq, bq),
                ("k", wk, bk_),
                ("v", wv, bv),
            ):
                w_sb[name] = const.tile(
                    [P, XC, CH], BF16, tag=f"w{name}", name=f"w{name}"
                )
                nc.sync.dma_start(
                    w_sb[name][:], wdram.rearrange("(o p) c -> p o c", p=P)
                )
                b_sb[name] = const.tile([CH, 1], F32, tag=f"b{name}", name=f"b{name}")
                nc.sync.dma_start(b_sb[name][:], bdram)
            wo_sb = const.tile([CH, D], BF16, tag="wo")
            mask_sb = const.tile([P, pat_w], BF16, tag="mpat")

            # V^T per (batch, k-block, head): 128-col padded [V | ones | pad]
            vaug = {}
            for b in range(B):
                t = persist.tile(
                    [P, NKB, HPC, P],
                    BF16,
                    tag=f"vaug{b}",
                    name=f"vaug{b}",
                )
                nc.gpsimd.memset(t[:, :, :, DK : DK + 1], 1.0)
                vaug[b] = t

            qt, kt, vt = {}, {}, {}
            for b in range(B):
                for name, dst in (("k", kt), ("q", qt), ("v", vt)):
                    dst[b] = persist.tile(
                        [CH, S], BF16, tag=f"{name}t{b}", name=f"{name}t{b}"
                    )

            def project(b, name, xdram, tg):
                """One 512-token group of the q/k/v projection for batch b."""
                dst = {"q": qt, "k": kt, "v": vt}[name]
                g = b * NTG + tg
                xt = xtp.tile([P, XC, CW], BF16, tag="xt")
                for h in range(0, XC, 4):
                    nc.sync.dma_start(
                        xt[:, h : h + 4, :], xdram[g, :, h : h + 4, :]
                    )
                ps = pp.tile([CH, CW], F32, tag="pp")
                for xc in range(XC):
                    nc.tensor.matmul(
                        ps[:],
                        lhsT=w_sb[name][:, xc, :],
                        rhs=xt[:, xc, :],
                        start=(xc == 0),
                        stop=(xc == XC - 1),
                    )
                nc.vector.tensor_add(
                    dst[b][:, tg * CW : (tg + 1) * CW],
                    ps[:],
                    b_sb[name][:, 0:1].to_broadcast((CH, CW)),
                )

            def vtrans(b, kb):
                """DMA XBAR transpose of one 128-token V block into vaug."""
                for hl in range(HPC):
                    nc.sync.dma_start(
                        vaug[b][:, kb, hl, 0:DK],
                        vt[b][hl * DK : (hl + 1) * DK, kb * P : (kb + 1) * P],
                        transpose=True,
                    )

            def oproj_col(tcol, yt):
                for mo2 in range(MO // 2):
                    ob = obp.tile([P, 2, CW], BF16, tag="ob")
                    for half in range(2):
                        mo = 2 * mo2 + half
                        op_ps = pp.tile([P, CW], F32, tag="pp")
                        nc.tensor.matmul(
                            op_ps[:],
                            lhsT=wo_sb[:, mo * P : (mo + 1) * P],
                            rhs=yt[:],
                            start=True,
                            stop=True,
                        )
                        if half == 0:
                            nc.scalar.copy(ob[:, half, :], op_ps[:])
                        else:
                            nc.vector.tensor_copy(ob[:, half, :], op_ps[:])
                    nc.gpsimd.dma_start(
                        out[tcol, :, 2 * mo2 : 2 * mo2 + 2, :], ob[:]
                    )

            def attention_col(b, j):
                blocks = plan[j]
                q0 = j * CW
                yt = ytp.tile([CH, CW], BF16, tag="yt")
                if not blocks:
                    nc.gpsimd.memset(yt[:], 0.0)
                    return yt
                ops = {}
                for hl in range(HPC):
                    ops[hl] = opp.tile([DK + 1, CW], F32, tag="op", name=f"op{hl}")
                nblk = len(blocks)

                def emit_av(i, bk, a2s, qlo, qhi):
                    for hl in range(HPC):
                        nc.tensor.matmul(
                            ops[hl][:, qlo:qhi],
                            lhsT=vaug[b][:, bk, hl, 0 : DK + 1],
                            rhs=a2s[hl][:, qlo:qhi],
                            start=(i == 0),
                            stop=(i == nblk - 1),
                        )

                # software pipeline: AV lags one block behind S/exp so the
                # exp latency hides behind the next block's S matmuls
                pend_av = None
                for i, (bk, qlo, qhi, mixed) in enumerate(blocks):
                    k0 = bk * P
                    a2s = {}
                    for hl in range(HPC):
                        hs = slice(hl * DK, (hl + 1) * DK)
                        s2 = s2p.tile([P, CW], F32, tag="s2", name=f"s2_{hl}")
                        nc.tensor.matmul(
                            s2[:, qlo:qhi],
                            lhsT=kt[b][hs, k0 : k0 + P],
                            rhs=qt[b][hs, q0 + qlo : q0 + qhi],
                            start=True,
                            stop=True,
                        )
                        a2 = a2p.tile([P, CW], BF16, tag="a2", name=f"a2_{hl}")
                        nc.scalar.activation(
                            a2[:, qlo:qhi],
                            s2[:, qlo:qhi],
                            mybir.ActivationFunctionType.Exp,
                            scale=0.125,
                        )
                        if mixed is not None:
                            off, a_, w_ = mixed
                            nc.vector.tensor_tensor(
                                a2[:, a_ : a_ + w_],
                                a2[:, a_ : a_ + w_],
                                mask_sb[:, off : off + w_],
                                mybir.AluOpType.mult,
                            )
                        a2s[hl] = a2
                    if pend_av is not None:
                        emit_av(*pend_av)
                    pend_av = (i, bk, a2s, qlo, qhi)
                emit_av(*pend_av)
                # reciprocal of the sums rows -> bf16 -> broadcast across the
                # head's 64 partitions via a K=1 outer-product matmul, then
                # normalize straight out of PSUM into yt
                sc = scp.tile([P, CW], F32, tag="sc")
                for hl in range(HPC):
                    sums = small.tile([1, CW], F32, tag="sums", name=f"sums{hl}")
                    nc.vector.tensor_copy(sums[:], ops[hl][DK : DK + 1, :])
                    recf = small.tile([1, CW], F32, tag="recf", name=f"recf{hl}")
                    nc.vector.reciprocal_approx_fast(out=recf[:], in_=sums[:])
                    recb = small.tile([1, CW], BF16, tag="recb", name=f"recb{hl}")
                    nc.scalar.copy(recb[:], recf[:])
                    nc.tensor.matmul(
                        sc[hl * DK : (hl + 1) * DK, :],
                        lhsT=ones64[:],
                        rhs=recb[:],
                        start=True,
                        stop=True,
                    )
                scale_sb = small.tile([P, CW], BF16, tag="scsb")
                nc.scalar.copy(scale_sb[:], sc[:])
                for hl in range(HPC):
                    nc.vector.tensor_tensor(
                        yt[hl * DK : (hl + 1) * DK, :],
                        ops[hl][0:DK, :],
                        scale_sb[hl * DK : (hl + 1) * DK, :],
                        mybir.AluOpType.mult,
                    )
                return yt

            pending = []
            for slot in range(n_slots):
                b, tg = divmod(slot, NTG)
                project(b, "k", xk, tg)
                project(b, "v", xv, tg)
                project(b, "q", xq, tg)
                if slot == 0:
                    nc.sync.dma_start(mask_sb[:], mpat)
                    nc.sync.dma_start(wo_sb[:], wo)
                for kb in range(tg * KPG, (tg + 1) * KPG):
                    vtrans(b, kb)
                for tcol, yt in pending:
                    oproj_col(tcol, yt)
                pending = []
                for ab, aj in attn_sched[slot]:
                    yt = attention_col(ab, aj)
                    pending.append((ab * NJ + aj, yt))
            for ab, aj in attn_sched[n_slots]:
                yt = attention_col(ab, aj)
                pending.append((ab * NJ + aj, yt))
            for tcol, yt in pending:
                oproj_col(tcol, yt)
    nc.compile()
    return nc


def _get_module(plan, pat_w):
    key = (plan, pat_w)
    if key not in _BUILD_CACHE:
        _BUILD_CACHE[key] = _build(plan, pat_w)
    return _BUILD_CACHE[key]


def _prep_inputs(query, key, value, mask, W_q, b_q, W_k, b_k, W_v, b_v, W_o, b_o):
    def xt_of(x):
        x2 = np.asarray(x, np.float32).reshape(TOK, D)
        xt = x2.T.astype(NPBF16)  # (D, TOK)
        xt = xt.reshape(XC, P, B * NTG, CW).transpose(2, 1, 0, 3)
        return np.ascontiguousarray(xt)  # (NTT, P, XC, CW)

    xq, xk, xv = xt_of(query), xt_of(key), xt_of(value)
    plan, pat_arr = _analyze_mask(mask)
    mpat = np.ascontiguousarray(pat_arr).astype(NPBF16)

    W_q = np.asarray(W_q, np.float32)
    W_k = np.asarray(W_k, np.float32)
    W_v = np.asarray(W_v, np.float32)
    W_o = np.asarray(W_o, np.float32)

    in_maps = []
    for c in range(N_CORES):
        cs = slice(c * CH, (c + 1) * CH)
        in_maps.append(
            {
                "xq": xq,
                "xk": xk,
                "xv": xv,
                "wq": np.ascontiguousarray(W_q[cs, :].T).astype(NPBF16),
                "wk": np.ascontiguousarray(W_k[cs, :].T).astype(NPBF16),
                "wv": np.ascontiguousarray(W_v[cs, :].T).astype(NPBF16),
                "wo": np.ascontiguousarray(W_o[:, cs].T).astype(NPBF16),
                "bq": np.asarray(b_q, np.float32)[cs].reshape(CH, 1).copy(),
                "bk": np.asarray(b_k, np.float32)[cs].reshape(CH, 1).copy(),
                "bv": np.asarray(b_v, np.float32)[cs].reshape(CH, 1).copy(),
                "mpat": mpat,
            }
        )
    return plan, mpat.shape[1], in_maps


def run(inputs, trace=False, trace_cores=None):
    """Build (cached), run on 8 cores, return (final_output, BassKernelResults)."""
    plan, pat_w, in_maps = _prep_inputs(**inputs)
    nc = _get_module(plan, pat_w)
    res = bass_utils.run_bass_kernel_spmd(
        nc,
        in_maps,
        core_ids=list(range(N_CORES)),
        trace=trace,
        trace_cores=trace_cores,
    )
    acc = np.zeros((B * NJ, P, MO, CW), np.float32)
    for c in range(N_CORES):
        acc += res.results[c]["out"].astype(np.float32)
    acc = acc.transpose(2, 1, 0, 3).reshape(D, TOK)
    final = acc.T + np.asarray(inputs["b_o"], np.float32)[None, :]
    return final.reshape(B, S, D), res


def kernel(**inputs):
    return run(inputs, trace=False)[0]


# revision 20
# speedup vs baseline: 1.2641x; 1.2641x over previous
"""Multi-head attention (B=2, S=2048, D=1024, H=16 heads, causal) on 8 trn2 cores.

Sharding: heads across cores (2 heads = 128 channels per core).
  - W_q/W_k/W_v column-sharded: each core projects all tokens to its 128 channels.
  - Attention per (batch, head) fully local to a core.
  - W_o row-sharded: each core computes a partial output projection; partials
    are summed on the host (the unshard step), then b_o is added.

Device layout: everything transposed (channels on partitions, tokens on free).
  - Scores computed as S^T blocks [128 k-tok, 512 q-tok] so exp is elementwise
    and the softmax sum comes for free from a ones-column packed next to V^T.
  - V^T is produced by DMA XBAR transpose (no PE/DVE involvement) into a
    per-(k-block, head) 128-column padded slot [V | ones | pad] (XBAR needs
    aligned destinations), so each head's AV lhsT is one contiguous
    65-column slice with the softmax sum landing in PSUM row 64.
  - Causal structure: host inspects the mask and emits only non-empty blocks,
    trimmed to their valid q-window [qlo, qhi); partially-valid q ranges are
    multiplied by a deduplicated 0/1 pattern tile post-exp.
  - Softmax normalization: reciprocal of the sums rows, broadcast across the
    64 partitions of each head via a K=1 outer-product matmul on the PE
    (no DRAM bounce), then one multiply per head straight out of PSUM.
  - Attention columns are emitted one 512-token projection group later than
    their inputs require, so the scheduler always has projection matmuls
    available to hide the exp/normalization latency and keep the PE clock warm.

All matmuls run in bf16 (inputs cast on host) with fp32 PSUM accumulation;
the partial output is returned bf16 and reduced in fp32 on the host.
"""

import sys

import numpy as np

try:
    import concourse.bass as bass  # noqa: F401
except ImportError:  # pragma: no cover
    sys.path.insert(0, "/opt/trn_rl_repo")

import ml_dtypes

import concourse.mybir as mybir
import concourse.tile as tile
from concourse import bacc, bass_utils
from concourse.masks import make_identity

P = 128
B, S, D = 2, 2048, 1024
H, DK = 16, 64
N_CORES = 8
HPC = H // N_CORES  # heads per core = 2
CH = HPC * DK  # channels per core = 128
TOK = B * S  # 4096
NKB = S // P  # k-blocks per batch = 16
CW = 512  # q column width
NJ = S // CW  # q columns per batch = 4
NTG = S // CW  # 512-token projection groups per batch = 4
KPG = CW // P  # k-blocks per token group = 4
XC = D // P  # x-dim chunks = 8
MO = D // P  # output-channel chunks = 8
VW = DK + 1  # 65: [V | ones] per head per k-block

BF16 = mybir.dt.bfloat16
F32 = mybir.dt.float32
NPBF16 = ml_dtypes.bfloat16

_BUILD_CACHE = {}


def _analyze_mask(mask):
    """Block plan from the (1,1,S,S) boolean mask (shared across batch/head).

    plan[j] = tuple of (bk, qlo, qhi, mixed) for each k-block with any valid
    entry; [qlo, qhi) is the smallest q-window containing all valid rows,
    mixed = (pat_off, a, w) for a range needing a 0/1 multiply (None if the
    window is fully valid). Patterns are deduplicated and concatenated into
    pats (P, W_total) in [k, q] layout.
    """
    m = np.asarray(mask).reshape(S, S).astype(bool)  # m[q, k]
    pat_index = {}
    pat_list = []
    plan = []
    for j in range(NJ):
        q0 = j * CW
        blocks = []
        for bk in range(NKB):
            sub = m[q0 : q0 + CW, bk * P : (bk + 1) * P]  # (CW q, P k)
            any_row = sub.any(axis=1)
            if not any_row.any():
                continue
            qlo = int(np.argmax(any_row))
            qhi = int(CW - np.argmax(any_row[::-1]))
            valid_all = sub.all(axis=1)
            notfull = ~valid_all
            notfull[:qlo] = False
            notfull[qhi:] = False
            mixed = None
            if notfull.any():
                idx = np.where(notfull)[0]
                a, b_ = int(idx[0]), int(idx[-1]) + 1
                patt = np.ascontiguousarray(sub[a:b_, :].T).astype(np.float32)
                key = (patt.shape[1], patt.tobytes())
                if key not in pat_index:
                    pat_index[key] = len(pat_list)
                    pat_list.append(patt)
                mixed = (pat_index[key], a, b_ - a)
            blocks.append((bk, qlo, qhi, mixed))
        plan.append(tuple(blocks))
    offs = [0]
    for p_ in pat_list:
        offs.append(offs[-1] + p_.shape[1])
    plan2 = []
    for col in plan:
        col2 = []
        for bk, qlo, qhi, mixed in col:
            if mixed is not None:
                pid, a, w = mixed
                mixed = (offs[pid], a, w)
            col2.append((bk, qlo, qhi, mixed))
        plan2.append(tuple(col2))
    if pat_list:
        pat_arr = np.concatenate(pat_list, axis=1)  # (P, W_total)
    else:
        pat_arr = np.ones((P, 1), np.float32)
    return tuple(plan2), pat_arr


def _build(plan, pat_w):
    nc = bacc.Bacc(
        "TRN2",
        target_bir_lowering=False,
        debug=False,
        enable_asserts=True,
        num_devices=N_CORES,
    )
    NTT = B * NTG
    xq = nc.dram_tensor("xq", [NTT, P, XC, CW], BF16, kind="ExternalInput").ap()
    xk = nc.dram_tensor("xk", [NTT, P, XC, CW], BF16, kind="ExternalInput").ap()
    xv = nc.dram_tensor("xv", [NTT, P, XC, CW], BF16, kind="ExternalInput").ap()
    wq = nc.dram_tensor("wq", [D, CH], BF16, kind="ExternalInput").ap()
    wk = nc.dram_tensor("wk", [D, CH], BF16, kind="ExternalInput").ap()
    wv = nc.dram_tensor("wv", [D, CH], BF16, kind="ExternalInput").ap()
    wo = nc.dram_tensor("wo", [CH, D], BF16, kind="ExternalInput").ap()
    bq = nc.dram_tensor("bq", [CH, 1], F32, kind="ExternalInput").ap()
    bk_ = nc.dram_tensor("bk", [CH, 1], F32, kind="ExternalInput").ap()
    bv = nc.dram_tensor("bv", [CH, 1], F32, kind="ExternalInput").ap()
    mpat = nc.dram_tensor("mpat", [P, pat_w], BF16, kind="ExternalInput").ap()
    out = nc.dram_tensor(
        "out", [B * NJ, P, MO, CW], BF16, kind="ExternalOutput"
    ).ap()

    # slot (one per 512-token projection group) at which each attention
    # column runs: one group AFTER its last input group, so projection
    # matmuls are always available to hide softmax latency.
    attn_after = [max((bk for bk, _, _, _ in col), default=0) // KPG for col in plan]
    n_slots = B * NTG
    attn_sched = [[] for _ in range(n_slots + 1)]
    for b in range(B):
        for j in range(NJ):
            s = min(b * NTG + attn_after[j] + 1, n_slots)
            attn_sched[s].append((b, j))

    with tile.TileContext(nc) as tc:
        with (
            tc.tile_pool(name="const", bufs=1) as const,
            tc.tile_pool(name="persist", bufs=1) as persist,
            tc.tile_pool(name="xt", bufs=6) as xtp,
            tc.tile_pool(name="a2", bufs=4) as a2p,
            tc.tile_pool(name="yt", bufs=4) as ytp,
            tc.tile_pool(name="ob", bufs=3) as obp,
            tc.tile_pool(name="small", bufs=3) as small,
            tc.tile_pool(name="pp", bufs=2, space="PSUM") as pp,
            tc.tile_pool(name="s2", bufs=3, space="PSUM") as s2p,
            tc.tile_pool(name="op", bufs=2, space="PSUM") as opp,
            tc.tile_pool(name="sc", bufs=1, space="PSUM") as scp,
        ):
            # ones row for the softmax-scale broadcast outer product:
            # scale[d, q] = ones64[0, d] * rec_h[0, q]
            ones64 = const.tile([1, DK], BF16, tag="ones64")
            nc.gpsimd.memset(ones64[:], 1.0)
            ident = const.tile([P, P], BF16, tag="ident")
            make_identity(nc, ident)

            w_sb = {}
            b_sb = {}
            for name, wdram, bdram in (
                ("q", wq, bq),
                ("k", wk, bk_),
                ("v", wv, bv),
            ):
                w_sb[name] = const.tile(
                    [P, XC, CH], BF16, tag=f"w{name}", name=f"w{name}"
                )
                nc.sync.dma_start(
                    w_sb[name][:], wdram.rearrange("(o p) c -> p o c", p=P)
                )
                b_sb[name] = const.tile([CH, 1], F32, tag=f"b{name}", name=f"b{name}")
                nc.sync.dma_start(b_sb[name][:], bdram)
            wo_sb = const.tile([CH, D], BF16, tag="wo")
            mask_sb = const.tile([P, pat_w], BF16, tag="mpat")

            # V^T per (batch, k-block, head): 128-col padded [V | ones | pad]
            vaug = {}
            for b in range(B):
                t = persist.tile(
                    [P, NKB, HPC, P],
                    BF16,
                    tag=f"vaug{b}",
                    name=f"vaug{b}",
                )
                nc.gpsimd.memset(t[:, :, :, DK : DK + 1], 1.0)
                vaug[b] = t

            qt, kt, vt = {}, {}, {}
            for b in range(B):
                for name, dst in (("k", kt), ("q", qt), ("v", vt)):
                    dst[b] = persist.tile(
                        [CH, S], BF16, tag=f"{name}t{b}", name=f"{name}t{b}"
                    )

            def project(b, name, xdram, tg):
                """One 512-token group of the q/k/v projection for batch b."""
                dst = {"q": qt, "k": kt, "v": vt}[name]
                g = b * NTG + tg
                xt = xtp.tile([P, XC, CW], BF16, tag="xt")
                nc.sync.dma_start(xt[:], xdram[g])
                ps = pp.tile([CH, CW], F32, tag="pp")
                for xc in range(XC):
                    nc.tensor.matmul(
                        ps[:],
                        lhsT=w_sb[name][:, xc, :],
                        rhs=xt[:, xc, :],
                        start=(xc == 0),
                        stop=(xc == XC - 1),
                    )
                nc.vector.tensor_add(
                    dst[b][:, tg * CW : (tg + 1) * CW],
                    ps[:],
                    b_sb[name][:, 0:1].to_broadcast((CH, CW)),
                )

            def vtrans(b, kb):
                """PE transpose of one 128-token V block into the vaug slots."""
                tp = pp.tile([P, P], BF16, tag="pp")
                nc.tensor.transpose(
                    tp[:], vt[b][:, kb * P : (kb + 1) * P], ident[:]
                )
                nc.vector.tensor_copy(
                    vaug[b][:, kb, :, 0:DK],
                    tp[:].rearrange("p (a b) -> p a b", a=HPC),
                )

            def oproj_col(tcol, yt):
                for mo2 in range(MO // 2):
                    ob = obp.tile([P, 2, CW], BF16, tag="ob")
                    for half in range(2):
                        mo = 2 * mo2 + half
                        op_ps = pp.tile([P, CW], F32, tag="pp")
                        nc.tensor.matmul(
                            op_ps[:],
                            lhsT=wo_sb[:, mo * P : (mo + 1) * P],
                            rhs=yt[:],
                            start=True,
                            stop=True,
                        )
                        if half == 0:
                            nc.scalar.copy(ob[:, half, :], op_ps[:])
                        else:
                            nc.vector.tensor_copy(ob[:, half, :], op_ps[:])
                    nc.gpsimd.dma_start(
                        out[tcol, :, 2 * mo2 : 2 * mo2 + 2, :], ob[:]
                    )

            def attention_col(b, j):
                blocks = plan[j]
                q0 = j * CW
                yt = ytp.tile([CH, CW], BF16, tag="yt")
                if not blocks:
                    nc.gpsimd.memset(yt[:], 0.0)
                    return yt
                ops = {}
                for hl in range(HPC):
                    ops[hl] = opp.tile([DK + 1, CW], F32, tag="op", name=f"op{hl}")
                nblk = len(blocks)

                def emit_av(i, bk, a2s, qlo, qhi):
                    for hl in range(HPC):
                        nc.tensor.matmul(
                            ops[hl][:, qlo:qhi],
                            lhsT=vaug[b][:, bk, hl, 0 : DK + 1],
                            rhs=a2s[hl][:, qlo:qhi],
                            start=(i == 0),
                            stop=(i == nblk - 1),
                        )

                # software pipeline: AV lags one block behind S/exp so the
                # exp latency hides behind the next block's S matmuls
                pend_av = None
                for i, (bk, qlo, qhi, mixed) in enumerate(blocks):
                    k0 = bk * P
                    a2s = {}
                    for hl in range(HPC):
                        hs = slice(hl * DK, (hl + 1) * DK)
                        s2 = s2p.tile([P, CW], F32, tag="s2", name=f"s2_{hl}")
                        nc.tensor.matmul(
                            s2[:, qlo:qhi],
                            lhsT=kt[b][hs, k0 : k0 + P],
                            rhs=qt[b][hs, q0 + qlo : q0 + qhi],
                            start=True,
                            stop=True,
                        )
                        a2 = a2p.tile([P, CW], BF16, tag="a2", name=f"a2_{hl}")
                        nc.scalar.activation(
                            a2[:, qlo:qhi],
                            s2[:, qlo:qhi],
                            mybir.ActivationFunctionType.Exp,
                            scale=0.125,
                        )
                        if mixed is not None:
                            off, a_, w_ = mixed
                            nc.gpsimd.tensor_tensor(
                                a2[:, a_ : a_ + w_],
                                a2[:, a_ : a_ + w_],
                                mask_sb[:, off : off + w_],
                                mybir.AluOpType.mult,
                            )
                        a2s[hl] = a2
                    if pend_av is not None:
                        emit_av(*pend_av)
                    pend_av = (i, bk, a2s, qlo, qhi)
                emit_av(*pend_av)
                # reciprocal of the sums rows -> bf16 -> broadcast across the
                # head's 64 partitions via a K=1 outer-product matmul, then
                # normalize straight out of PSUM into yt
                sc = scp.tile([P, CW], F32, tag="sc")
                for hl in range(HPC):
                    sums = small.tile([1, CW], F32, tag="sums", name=f"sums{hl}")
                    nc.vector.tensor_copy(sums[:], ops[hl][DK : DK + 1, :])
                    recf = small.tile([1, CW], F32, tag="recf", name=f"recf{hl}")
                    nc.vector.reciprocal_approx_fast(out=recf[:], in_=sums[:])
                    recb = small.tile([1, CW], BF16, tag="recb", name=f"recb{hl}")
                    nc.scalar.copy(recb[:], recf[:])
                    nc.tensor.matmul(
                        sc[hl * DK : (hl + 1) * DK, :],
                        lhsT=ones64[:],
                        rhs=recb[:],
                        start=True,
                        stop=True,
                    )
                scale_sb = small.tile([P, CW], BF16, tag="scsb")
                nc.scalar.copy(scale_sb[:], sc[:])
                for hl in range(HPC):
                    nc.vector.tensor_tensor(
                        yt[hl * DK : (hl + 1) * DK, :],
                        ops[hl][0:DK, :],
                        scale_sb[hl * DK : (hl + 1) * DK, :],
                        mybir.AluOpType.mult,
                    )
                return yt

            pending = []
            for slot in range(n_slots):
                b, tg = divmod(slot, NTG)
                project(b, "k", xk, tg)
                project(b, "v", xv, tg)
                project(b, "q", xq, tg)
                if slot == 0:
                    nc.sync.dma_start(mask_sb[:], mpat)
                    nc.sync.dma_start(wo_sb[:], wo)
                for kb in range(tg * KPG, (tg + 1) * KPG):
                    vtrans(b, kb)
                for tcol, yt in pending:
                    oproj_col(tcol, yt)
                pending = []
                for ab, aj in attn_sched[slot]:
                    yt = attention_col(ab, aj)
                    pending.append((ab * NJ + aj, yt))
            for ab, aj in attn_sched[n_slots]:
                yt = attention_col(ab, aj)
                pending.append((ab * NJ + aj, yt))
            for tcol, yt in pending:
                oproj_col(tcol, yt)
    nc.compile()
    return nc


def _get_module(plan, pat_w):
    key = (plan, pat_w)
    if key not in _BUILD_CACHE:
        _BUILD_CACHE[key] = _build(plan, pat_w)
    return _BUILD_CACHE[key]


def _prep_inputs(query, key, value, mask, W_q, b_q, W_k, b_k, W_v, b_v, W_o, b_o):
    def xt_of(x):
        x2 = np.asarray(x, np.float32).reshape(TOK, D)
        xt = x2.T.astype(NPBF16)  # (D, TOK)
        xt = xt.reshape(XC, P, B * NTG, CW).transpose(2, 1, 0, 3)
        return np.ascontiguousarray(xt)  # (NTT, P, XC, CW)

    xq, xk, xv = xt_of(query), xt_of(key), xt_of(value)
    plan, pat_arr = _analyze_mask(mask)
    mpat = np.ascontiguousarray(pat_arr).astype(NPBF16)

    W_q = np.asarray(W_q, np.float32)
    W_k = np.asarray(W_k, np.float32)
    W_v = np.asarray(W_v, np.float32)
    W_o = np.asarray(W_o, np.float32)

    in_maps = []
    for c in range(N_CORES):
        cs = slice(c * CH, (c + 1) * CH)
        in_maps.append(
            {
                "xq": xq,
                "xk": xk,
                "xv": xv,
                "wq": np.ascontiguousarray(W_q[cs, :].T).astype(NPBF16),
                "wk": np.ascontiguousarray(W_k[cs, :].T).astype(NPBF16),
                "wv": np.ascontiguousarray(W_v[cs, :].T).astype(NPBF16),
                "wo": np.ascontiguousarray(W_o[:, cs].T).astype(NPBF16),
                "bq": np.asarray(b_q, np.float32)[cs].reshape(CH, 1).copy(),
                "bk": np.asarray(b_k, np.float32)[cs].reshape(CH, 1).copy(),
                "bv": np.asarray(b_v, np.float32)[cs].reshape(CH, 1).copy(),
                "mpat": mpat,
            }
        )
    return plan, mpat.shape[1], in_maps


def run(inputs, trace=False, trace_cores=None):
    """Build (cached), run on 8 cores, return (final_output, BassKernelResults)."""
    plan, pat_w, in_maps = _prep_inputs(**inputs)
    nc = _get_module(plan, pat_w)
    res = bass_utils.run_bass_kernel_spmd(
        nc,
        in_maps,
        core_ids=list(range(N_CORES)),
        trace=trace,
        trace_cores=trace_cores,
    )
    acc = np.zeros((B * NJ, P, MO, CW), np.float32)
    for c in range(N_CORES):
        acc += res.results[c]["out"].astype(np.float32)
    acc = acc.transpose(2, 1, 0, 3).reshape(D, TOK)
    final = acc.T + np.asarray(inputs["b_o"], np.float32)[None, :]
    return final.reshape(B, S, D), res


def kernel(**inputs):
    return run(inputs, trace=False)[0]
